# Initial kernel scaffold
#
"""Trainium2 Bass kernel for nn_Encoder_Block (B=2,S=2048,E=1024,H=16,D=64,FE=4).

Sharding: 8 NeuronCores, no collectives. Cores 0-3 take batch 0, cores 4-7
batch 1; each core owns a 512-query slice and runs the full encoder block
for those queries (it loads all keys/values of its batch plus all weights).

Per-core pipeline, per head:
  kT slice --fp32r--> k' = Aqk.T @ kT          (folds Wq,Wk into keys; PE fp32r)
  pass1: scores[q,k] = qT.T @ k'  -> row max m via DVE reduce_max from PSUM
  pass2: scoresT[k,q] = k'_aug.T @ qT_aug      (65th row subtracts m in-matmul)
         -> one ACT pass: exp(sqrt(S)*x) PSUM->SBUF bf16  = attnT
  ov: v_aug.T @ attnT accumulated over k-tiles ([65,q]; row 64 = sum(exp) = Z)
      -> multiply by 1/Z during drain (Zinv broadcast via DRAM bounce)
Then fc (Wv folded into Wfc), residual + LN1 (bn_stats), FFN1 + relu(+b1 via
ACT bias), FFN2 (+b2 via K=1 matmul), residual + LN2.  Weights are
pre-transposed / pre-cast / pre-tiled on the host; q/k/v are host-transposed.
"""
import os
import sys
import math
from contextlib import ExitStack

os.environ.setdefault("NEURON_RT_RESET_CORES", "1")
sys.path.insert(0, "/opt/trn_rl_repo")

import numpy as np
import concourse.bass as bass
import concourse.tile as tile
from concourse import mybir

F32 = mybir.dt.float32
F32R = mybir.dt.float32r
BF16 = mybir.dt.bfloat16
AX = mybir.AxisListType.X
AF = mybir.ActivationFunctionType
OP = mybir.AluOpType


class Cfg:
    def __init__(self, S=2048, E=1024, H=16, D=64, FE=4, T=512, eps=1e-5):
        self.S, self.E, self.H, self.D, self.FE, self.T, self.eps = S, E, H, D, FE, T, eps
        assert D == 64 and E == H * D
        self.KT = S // 128            # k partition-tiles
        self.QT = T // 128            # q tiles (per core)
        self.ET = E // 128            # e tiles
        self.ZT = FE * E // 128       # ffn hidden tiles
        self.CH = min(512, S)         # k moving chunk for pass1 / k'
        self.NCH = S // self.CH
        self.EC = min(512, E)         # e moving chunk
        self.NEC = E // self.EC
        self.P2B = 2 if self.KT % 2 == 0 else 1   # pass-2 k-tiles per exp batch
        self.scale = math.sqrt(float(S))

    def perm(self):
        # pass-2 query order j <-> original query (j % QT)*128 + j // QT
        j = np.arange(self.T)
        return (j % self.QT) * 128 + j // self.QT


def _layernorm(nc, pool, x_ap, out_ap, g_b, b_b, eps_t, c, out_dtype=None):
    """LayerNorm over the free dim (E) of x_ap [128, E] -> out_ap."""
    E = c.E
    nsub = (E + 511) // 512
    sub = E // nsub
    stats = pool.tile([128, nsub, 6], F32, tag="ln_stats")
    xr = x_ap.rearrange("p (n s) -> p n s", n=nsub)
    for i in range(nsub):
        nc.vector.bn_stats(stats[:, i, :], xr[:, i, :])
    mv = pool.tile([128, 2], F32, tag="ln_mv")
    nc.vector.bn_aggr(mv[:], stats[:])
    rstd = pool.tile([128, 1], F32, tag="ln_rstd")
    nc.scalar.activation(rstd[:], mv[:, 1:2], AF.Sqrt, bias=eps_t[:], scale=1.0)
    nc.vector.reciprocal(rstd[:], rstd[:])
    t1 = pool.tile([128, E], F32, tag="ln_t1")
    nc.vector.scalar_tensor_tensor(
        t1[:], x_ap, mv[:, 0:1], rstd[:].to_broadcast([128, E]),
        OP.subtract, OP.mult)
    t2 = pool.tile([128, E], F32, tag="ln_t2")
    nc.vector.tensor_tensor(t2[:], t1[:], g_b[:], OP.mult)
    nc.vector.tensor_tensor(out_ap, t2[:], b_b[:], OP.add)


def build_nc(c: Cfg):
    """Build the single-core program (pure SPMD — all cores run this)."""
    nc = bass.Bass()
    S, E, H, D, T = c.S, c.E, c.H, c.D, c.T

    dp = nc.declare_dram_parameter
    # k split hi/lo, interleaved per head: [H][0:64]=kh^T, [64:128]=kl^T
    khl_d = dp("khl", [H, 128, S], F32, isOutput=False)
    qh_d = dp("qh", [E, T], F32, isOutput=False)             # q'_hi^T (orig order)
    qhd_d = dp("qhd", [H, 128, T], F32, isOutput=False)      # q'_hi^T dup (perm order)
    ql_d = dp("ql", [E, T], F32, isOutput=False)             # q'_lo^T (perm order)
    qnat_d = dp("qnat", [T, E], F32, isOutput=False)         # queries rows (perm order)
    v_d = dp("vv", [S, E], BF16, isOutput=False)             # values of batch
    wfc_d = dp("wfc", [128, c.ET, E], BF16, isOutput=False)  # Wfc_v^T tiled
    bfc_d = dp("bfc", [1, E], BF16, isOutput=False)
    w1_d = dp("w1", [c.ZT, 128, E], BF16, isOutput=False)    # per zt: [e_in part, z cols]
    b1_d = dp("b1", [128, c.ZT], F32, isOutput=False)
    w2_d = dp("w2", [c.ZT, 128, E], BF16, isOutput=False)    # per zt: [z part, e cols]
    b2_d = dp("b2", [1, E], BF16, isOutput=False)
    g1_d = dp("g1", [1, E], F32, isOutput=False)
    be1_d = dp("be1", [1, E], F32, isOutput=False)
    g2_d = dp("g2", [1, E], F32, isOutput=False)
    be2_d = dp("be2", [1, E], F32, isOutput=False)
    out_d = dp("out", [T, E], F32, isOutput=True)            # perm rows

    with tile.TileContext(nc) as tc, ExitStack() as ctx:
        persist = ctx.enter_context(tc.tile_pool(name="persist", bufs=1))

        def bcast128(src_ap, nm, dtype=F32):
            t = persist.tile([128, src_ap.shape[1]], dtype, name=nm, tag=nm)
            src_b = bass.AP(tensor=src_ap.tensor, offset=src_ap.offset,
                            ap=[[0, 128]] + list(src_ap.ap[1:]))
            nc.sync.dma_start(t[:], src_b)
            return t

        g1_b = bcast128(g1_d[:], "g1b")
        be1_b = bcast128(be1_d[:], "be1b")
        g2_b = bcast128(g2_d[:], "g2b")
        be2_b = bcast128(be2_d[:], "be2b")

        from concourse.masks import make_identity
        ident = persist.tile([128, 128], BF16)
        make_identity(nc, ident[:])

        eps_t = persist.tile([128, 1], F32)
        nc.vector.memset(eps_t[:], c.eps)

        ones_f = persist.tile([1, 512], F32)
        nc.vector.memset(ones_f[:], 1.0)
        ones_bf = persist.tile([1, 128], BF16)
        nc.vector.memset(ones_bf[:], 1.0)

        wfc_t = persist.tile([128, c.ET, E], BF16)
        nc.sync.dma_start(wfc_t[:], wfc_d[:])
        bfc_t = persist.tile([1, E], BF16)
        nc.sync.dma_start(bfc_t[:], bfc_d[:])
        b1_t = persist.tile([128, c.ZT], F32)
        nc.sync.dma_start(b1_t[:], b1_d[:])
        b2_t = persist.tile([1, E], BF16)
        nc.sync.dma_start(b2_t[:], b2_d[:])

        ovT_pack = persist.tile([128, c.ET, T], BF16)
        h_sb = persist.tile([128, c.QT, E], F32)
        hT_bf = persist.tile([128, c.ET, T], BF16)
        x_sb = persist.tile([128, c.QT, E], F32)

        # =================== ATTENTION ===================
        with ExitStack() as actx:
            kst_p = actx.enter_context(tc.tile_pool(name="kst", bufs=1))
            kaug_p = actx.enter_context(tc.tile_pool(name="kaug", bufs=2))
            qtp_p = actx.enter_context(tc.tile_pool(name="qtp", bufs=2))
            qaug_p = actx.enter_context(tc.tile_pool(name="qaug", bufs=2))
            ovs_p = actx.enter_context(tc.tile_pool(name="ovs", bufs=2))
            attn_p = actx.enter_context(tc.tile_pool(name="attn", bufs=2))
            vv_p = actx.enter_context(tc.tile_pool(name="vv", bufs=2))
            sm_p = actx.enter_context(tc.tile_pool(name="sm", bufs=3))
            zi_p = actx.enter_context(tc.tile_pool(name="zi", bufs=2))
            zdr_p = actx.enter_context(tc.tile_pool(name="zdr", bufs=2, space="DRAM"))
            mm_ps = actx.enter_context(tc.tile_pool(name="mm_ps", bufs=2, space="PSUM"))
            p2_ps = actx.enter_context(tc.tile_pool(name="p2_ps", bufs=2, space="PSUM"))
            ov_ps = actx.enter_context(tc.tile_pool(name="ov_ps", bufs=2, space="PSUM"))

            ovst = None
            for h in range(H):
                if h % 2 == 0:
                    ovst = ovs_p.tile([64, 2, T], BF16, tag="ovst")

                # ---- per-head q'_hi rounding (orig order, pass-1 lhsT) ----
                qtstg = qtp_p.tile([64, T], F32, tag="qtstg")
                nc.sync.dma_start(qtstg[:], qh_d[h * D:(h + 1) * D, :])
                qtr = qtp_p.tile([64, T], F32R, tag="qtr")
                nc.vector.tensor_copy(qtr[:], qtstg[:])

                # ---- k hi/lo staging + rounding ----
                k_stage = kst_p.tile([128, S], F32, tag="kst")
                nc.sync.dma_start(k_stage[:], khl_d[h, :, :])
                khl_r = kst_p.tile([128, S], F32R, tag="khlr")
                nc.vector.tensor_copy(khl_r[:], k_stage[:])

                # lhsT-B: [kh; ones] [65, S]
                kaug = kaug_p.tile([65, S], F32R, tag="kaug")
                nc.vector.tensor_copy(kaug[:64, :], k_stage[:64, :])
                nc.scalar.copy(kaug[64:65, :],
                               ones_f[:, 0:1].to_broadcast([1, S]))  # ones row

                # rhs-A: q'_hi duplicated [128, T] (perm order)
                qdstg = qtp_p.tile([128, T], F32, tag="qdstg")
                nc.sync.dma_start(qdstg[:], qhd_d[h, :, :])
                qdup_r = qtp_p.tile([128, T], F32R, tag="qdup")
                nc.vector.tensor_copy(qdup_r[:], qdstg[:])

                # ---- pass 1: per-row max ----
                m_neg = sm_p.tile([128, c.QT], F32, tag="mneg")
                for qt in range(c.QT):
                    mtmp = sm_p.tile([128, max(c.NCH, 2)], F32, tag="mtmp")
                    for j in range(c.NCH):
                        sl = slice(j * c.CH, (j + 1) * c.CH)
                        sps = mm_ps.tile([128, c.CH], F32, tag="mmps")
                        nc.tensor.matmul(
                            sps[:], qtr[:, qt * 128:(qt + 1) * 128],
                            khl_r[:64, sl], start=True, stop=True)
                        nc.vector.reduce_max(mtmp[:, j:j + 1], sps[:], axis=AX)
                    nc.vector.reduce_max(m_neg[:, qt:qt + 1], mtmp[:, :c.NCH], axis=AX)
                nc.vector.tensor_scalar_mul(m_neg[:], m_neg[:], -1.0)

                # ---- qT_aug [65, T]: perm-order queries + (-m) aug row ----
                # (m [128, QT] -> [1, T] free-dim row needs a DRAM bounce:
                #  SBUF APs cannot flatten across partitions)
                m_dram = zdr_p.tile([128, c.QT], F32, tag="mdram")
                nc.sync.dma_start(m_dram[:], m_neg[:])
                q_stage = qaug_p.tile([65, T], F32, tag="qstage")
                nc.sync.dma_start(q_stage[:64, :], ql_d[h * D:(h + 1) * D, :])
                nc.sync.dma_start(q_stage[64:65, :],
                                  m_dram[:].rearrange("r qt -> (r qt)")[None, :])
                qaug = qaug_p.tile([65, T], F32R, tag="qaug")
                nc.vector.tensor_copy(qaug[:], q_stage[:])

                # ---- v_aug [128, KT, 65] (ones column for Z) ----
                vaug = vv_p.tile([128, c.KT, 65], BF16, tag="vaug")
                nc.sync.dma_start(
                    vaug[:, :, :64],
                    v_d[:, h * D:(h + 1) * D].rearrange("(t p) d -> p t d", p=128))
                nc.vector.memset(vaug[:, :, 64:65], 1.0)

                # ---- pass 2 (scoresT - m), exp, ov ----
                attnT = attn_p.tile([128, c.KT, T], BF16, tag="attnT")
                ovp = ov_ps.tile([65, T], F32, tag="ovps")
                for tb in range(0, c.KT, c.P2B):
                    p2 = p2_ps.tile([128, c.P2B, T], F32, tag="p2ps")
                    for ti in range(c.P2B):
                        t = tb + ti
                        tsl = slice(t * 128, (t + 1) * 128)
                        # (kh+kl)*q'hi  then  kh*q'lo + (-m)
                        nc.tensor.matmul(p2[:, ti, :], khl_r[:, tsl], qdup_r[:],
                                         start=True, stop=False)
                        nc.tensor.matmul(p2[:, ti, :], kaug[:, tsl], qaug[:],
                                         start=False, stop=True)
                    nc.scalar.activation(attnT[:, tb:tb + c.P2B, :], p2[:],
                                         AF.Exp, bias=0.0, scale=c.scale)
                    for ti in range(c.P2B):
                        t = tb + ti
                        nc.tensor.matmul(
                            ovp[:], vaug[:, t, :], attnT[:, t, :],
                            start=(t == 0), stop=(t == c.KT - 1),
                            skip_group_check=True)

                # ---- 1/Z and ovT drain ----
                zrow = zi_p.tile([65, T], F32, tag="zrow")
                nc.vector.reciprocal(zrow[64:65, :], ovp[64:65, :])
                zdr = zdr_p.tile([1, T], F32, tag="zdr")
                nc.sync.dma_start(zdr[:], zrow[64:65, :])
                zinv_b = zi_p.tile([64, T], F32, tag="zinv")
                zsrc = zdr[:]
                nc.sync.dma_start(
                    zinv_b[:],
                    bass.AP(tensor=zsrc.tensor, offset=zsrc.offset,
                            ap=[[0, 64]] + list(zsrc.ap[1:])))
                nc.vector.scalar_tensor_tensor(
                    ovst[:, h % 2, :], ovp[:64, :], 1.0, zinv_b[:],
                    OP.bypass, OP.mult)

                if h % 2 == 1:
                    # pack pair -> ovT_pack [128, ET, T]
                    nc.sync.dma_start(ovT_pack[:64, h // 2, :], ovst[:, 0, :])
                    nc.sync.dma_start(ovT_pack[64:128, h // 2, :], ovst[:, 1, :])

        # =================== FC + LN1 + transpose(h) ===================
        with ExitStack() as fctx:
            qn_p = fctx.enter_context(tc.tile_pool(name="qn", bufs=2))
            st_p = fctx.enter_context(tc.tile_pool(name="st", bufs=2))
            fc_ps = fctx.enter_context(tc.tile_pool(name="fc_ps", bufs=2, space="PSUM"))
            tr_ps = fctx.enter_context(tc.tile_pool(name="tr_ps", bufs=2, space="PSUM"))

            for qt in range(c.QT):
                qsl = slice(qt * 128, (qt + 1) * 128)
                hpre = st_p.tile([128, E], F32, tag="hpre")
                qn = qn_p.tile([128, E], F32, tag="qn")
                nc.sync.dma_start(qn[:], qnat_d[qsl, :])
                for ec in range(c.NEC):
                    esl = slice(ec * c.EC, (ec + 1) * c.EC)
                    aps = fc_ps.tile([128, c.EC], F32, tag="fcps")
                    for dt in range(c.ET):
                        nc.tensor.matmul(aps[:], ovT_pack[:, dt, qsl],
                                         wfc_t[:, dt, esl],
                                         start=(dt == 0), stop=False)
                    nc.tensor.matmul(aps[:], ones_bf[:, :128], bfc_t[:, esl],
                                     start=False, stop=True)
                    nc.vector.scalar_tensor_tensor(
                        hpre[:, esl], aps[:], 1.0, qn[:, esl],
                        OP.bypass, OP.add)

                _layernorm(nc, st_p, hpre[:], h_sb[:, qt, :], g1_b, be1_b, eps_t, c)
                hbf = st_p.tile([128, E], BF16, tag="hbf")
                nc.vector.tensor_copy(hbf[:], h_sb[:, qt, :])
                for et in range(c.ET):
                    tps = tr_ps.tile([128, 128], BF16, tag="trps")
                    nc.tensor.transpose(tps[:], hbf[:, et * 128:(et + 1) * 128],
                                        ident[:])
                    nc.vector.tensor_copy(hT_bf[:, et, qsl], tps[:])

        # =================== FFN + LN2 ===================
        with ExitStack() as nctx:
            w_p = nctx.enter_context(tc.tile_pool(name="wstream", bufs=4))
            z_p = nctx.enter_context(tc.tile_pool(name="zrel", bufs=1))
            ln_p = nctx.enter_context(tc.tile_pool(name="lnp", bufs=2))
            z1_ps = nctx.enter_context(tc.tile_pool(name="z1_ps", bufs=2, space="PSUM"))
            x2_ps = nctx.enter_context(
                tc.tile_pool(name="x2_ps", bufs=c.QT, space="PSUM"))

            z1rel = z_p.tile([128, c.ZT, T], BF16, tag="z1rel")
            for zt in range(c.ZT):
                w1t = w_p.tile([128, E], BF16, tag="w1t")
                nc.sync.dma_start(w1t[:], w1_d[zt, :, :])
                zps = z1_ps.tile([128, T], F32, tag="z1ps")
                for et in range(c.ET):
                    nc.tensor.matmul(zps[:], w1t[:, et * 128:(et + 1) * 128],
                                     hT_bf[:, et, :],
                                     start=(et == 0), stop=(et == c.ET - 1))
                nc.scalar.activation(z1rel[:, zt, :], zps[:], AF.Relu,
                                     bias=b1_t[:, zt:zt + 1], scale=1.0)

            for ec in range(c.NEC):
                esl = slice(ec * c.EC, (ec + 1) * c.EC)
                xps = [x2_ps.tile([128, c.EC], F32, tag="x2ps", name=f"x2ps_{qt}")
                       for qt in range(c.QT)]
                for zt in range(c.ZT):
                    w2t = w_p.tile([128, c.EC], BF16, tag="w2t")
                    nc.sync.dma_start(w2t[:], w2_d[zt, :, esl])
                    for qt in range(c.QT):
                        nc.tensor.matmul(
                            xps[qt][:], z1rel[:, zt, qt * 128:(qt + 1) * 128],
                            w2t[:], start=(zt == 0), stop=False)
                for qt in range(c.QT):
                    nc.tensor.matmul(xps[qt][:], ones_bf[:, :128], b2_t[:, esl],
                                     start=False, stop=True)
                    nc.vector.scalar_tensor_tensor(
                        x_sb[:, qt, esl], xps[qt][:], 1.0, h_sb[:, qt, esl],
                        OP.bypass, OP.add)

            for qt in range(c.QT):
                outt = ln_p.tile([128, E], F32, tag="outt")
                _layernorm(nc, ln_p, x_sb[:, qt, :], outt[:], g2_b, be2_b, eps_t, c)
                nc.sync.dma_start(out_d[qt * 128:(qt + 1) * 128, :], outt[:])

    return nc


def _split_waits(nc, maxw=1):
    """walrus in this toolchain only accepts 1 sync-wait per instruction on
    several formats; move excess waits onto preceding same-engine NoOps."""
    ctr = 0
    for f in nc.m.functions:
        for bb in f.blocks:
            out = []
            for inst in bb.instructions:
                si = getattr(inst, "sync_info", None)
                if si is not None and si.on_wait and len(si.on_wait) > maxw:
                    waits = list(si.on_wait)
                    head, tail = waits[:-maxw], waits[-maxw:]
                    for i in range(0, len(head), maxw):
                        ctr += 1
                        out.append(mybir.InstNoOp(
                            name=f"waitsplit_{ctr}", engine=inst.engine,
                            ins=[], outs=[],
                            sync_info=mybir.SyncInfo(
                                on_wait=list(head[i:i + maxw]), on_update=[]),
                        ))
                    si.on_wait = tail
                out.append(inst)
            bb.instructions[:] = out


# ======================= host side =======================

def host_prep(c: Cfg, inputs, core):
    """Build the per-core input map (numpy only)."""
    B = inputs["queries"].shape[0]
    cores_per_batch = 8 // B if B <= 8 else 1
    b = core // cores_per_batch
    slot = core % cores_per_batch
    T = c.T
    perm = c.perm()

    q = np.asarray(inputs["queries"][b], np.float32)       # [S, E]
    k = np.asarray(inputs["keys"][b], np.float32)
    v = np.asarray(inputs["values"][b], np.float32)
    qs = q[slot * T:(slot + 1) * T]                        # [T, E]

    Wq = np.asarray(inputs["Wq"], np.float64)
    Wk = np.asarray(inputs["Wk"], np.float64)
    Wv = np.asarray(inputs["Wv"], np.float64)
    Wfc = np.asarray(inputs["Wfc"], np.float64)            # [E, E]
    W1 = np.asarray(inputs["W1"], np.float64)              # [FE*E, E]
    W2 = np.asarray(inputs["W2"], np.float64)              # [E, FE*E]

    def trunc10(x):
        # keep 10 mantissa bits -> exactly representable in fp32r
        u = x.astype(np.float32).view(np.uint32) & np.uint32(0xFFFFE000)
        return u.view(np.float32)

    # fold Wq/Wk into the queries: q'_h = q_h @ (Wq.T @ Wk); scores = q' @ k^T
    A_mid = Wq.T @ Wk
    E_, H_, D_ = c.E, c.H, c.D
    qp = np.empty((T, E_), np.float64)
    for h in range(H_):
        qp[:, h * D_:(h + 1) * D_] = qs[:, h * D_:(h + 1) * D_].astype(np.float64) @ A_mid
    qp = qp.astype(np.float32)
    qp_h = trunc10(qp)
    qp_l = (qp - qp_h).astype(np.float32)
    k_h = trunc10(k)
    k_l = (k - k_h).astype(np.float32)

    khl = np.empty((c.H, 128, c.S), np.float32)
    for h in range(H_):
        khl[h, :64] = k_h[:, h * D_:(h + 1) * D_].T
        khl[h, 64:] = k_l[:, h * D_:(h + 1) * D_].T

    qhT = np.ascontiguousarray(qp_h.T)                     # [E, T] orig order
    qhTp = qp_h[perm].T                                    # [E, T] perm order
    qhd = np.empty((c.H, 128, T), np.float32)
    for h in range(H_):
        qhd[h, :64] = qhTp[h * D_:(h + 1) * D_]
        qhd[h, 64:] = qhTp[h * D_:(h + 1) * D_]
    qlTp = np.ascontiguousarray(qp_l[perm].T)              # [E, T] perm order

    # Wfc_v[e, h*64+d] = sum_dd Wfc[e, h*64+dd] * Wv[dd, d]
    E, H, D = c.E, c.H, c.D
    wfcv = np.empty((E, E), np.float64)
    for h in range(H):
        wfcv[:, h * D:(h + 1) * D] = Wfc[:, h * D:(h + 1) * D] @ Wv
    # rhs tiles: wfc_prep[p, dt, e] = Wfc_v[e, dt*128+p]
    wfc_prep = np.ascontiguousarray(
        wfcv.T.reshape(c.ET, 128, E).transpose(1, 0, 2)).astype(ml_bf16())

    # w1_prep[zt, p, et*128 + z] = W1[zt*128+z, et*128+p]
    w1r = W1.reshape(c.ZT, 128, c.ET, 128)                 # [zt, z, et, p]
    w1_prep = np.ascontiguousarray(
        w1r.transpose(0, 3, 2, 1).reshape(c.ZT, 128, E)).astype(ml_bf16())

    # w2_prep[zt, p, e] = W2[e, zt*128+p]
    w2r = W2.T.reshape(c.ZT, 128, E)                       # [zt, p, e]
    w2_prep = np.ascontiguousarray(w2r).astype(ml_bf16())

    b1 = np.asarray(inputs["b1"], np.float32)
    b1_prep = np.ascontiguousarray(b1.reshape(c.ZT, 128).T)  # [128, ZT]

    return {
        "khl": khl,
        "qh": qhT,
        "qhd": qhd,
        "ql": qlTp,
        "qnat": np.ascontiguousarray(qs[perm]),
        "vv": v.astype(ml_bf16()),
        "wfc": wfc_prep,
        "bfc": np.asarray(inputs["bfc"], np.float32)[None, :].astype(ml_bf16()),
        "w1": w1_prep,
        "b1": b1_prep,
        "w2": w2_prep,
        "b2": np.asarray(inputs["b2"], np.float32)[None, :].astype(ml_bf16()),
        "g1": np.asarray(inputs["ln1_g"], np.float32)[None, :],
        "be1": np.asarray(inputs["ln1_b"], np.float32)[None, :],
        "g2": np.asarray(inputs["ln2_g"], np.float32)[None, :],
        "be2": np.asarray(inputs["ln2_b"], np.float32)[None, :],
    }


def ml_bf16():
    import ml_dtypes
    return ml_dtypes.bfloat16


_CACHE = {}


def kernel(**inputs):
    """Full-input entry point: shard across 8 cores, run, gather."""
    c = Cfg()
    B, S, E = inputs["queries"].shape
    assert (B, S, E) == (2, c.S, c.E), (B, S, E)

    if "nc" not in _CACHE:
        nc = build_nc(c)
        _split_waits(nc)   # walrus wait-slot workaround (compile path only)
        _CACHE["nc"] = nc
    nc = _CACHE["nc"]

    in_maps = [host_prep(c, inputs, core) for core in range(8)]

    from concourse.bass_utils import run_bass_kernel_spmd
    res = run_bass_kernel_spmd(nc, in_maps, core_ids=list(range(8)))

    perm = c.perm()
    out = np.empty((B, S, E), np.float32)
    cores_per_batch = 4
    for core in range(8):
        b = core // cores_per_batch
        slot = core % cores_per_batch
        block = np.empty((c.T, E), np.float32)
        block[perm] = res.results[core]["out"]
        out[b, slot * c.T:(slot + 1) * c.T] = block
    return out



# revision 1
# speedup vs baseline: 1.0102x; 1.0102x over previous
"""Trainium2 Bass kernel for nn_Encoder_Block (B=2,S=2048,E=1024,H=16,D=64,FE=4).

Sharding: 8 NeuronCores, no collectives. Cores 0-3 take batch 0, cores 4-7
batch 1; each core owns a 512-query slice and runs the full encoder block
for those queries (it loads all keys/values of its batch plus all weights).

Per-core pipeline, per head:
  kT slice --fp32r--> k' = Aqk.T @ kT          (folds Wq,Wk into keys; PE fp32r)
  pass1: scores[q,k] = qT.T @ k'  -> row max m via DVE reduce_max from PSUM
  pass2: scoresT[k,q] = k'_aug.T @ qT_aug      (65th row subtracts m in-matmul)
         -> one ACT pass: exp(sqrt(S)*x) PSUM->SBUF bf16  = attnT
  ov: v_aug.T @ attnT accumulated over k-tiles ([65,q]; row 64 = sum(exp) = Z)
      -> multiply by 1/Z during drain (Zinv broadcast via DRAM bounce)
Then fc (Wv folded into Wfc), residual + LN1 (bn_stats), FFN1 + relu(+b1 via
ACT bias), FFN2 (+b2 via K=1 matmul), residual + LN2.  Weights are
pre-transposed / pre-cast / pre-tiled on the host; q/k/v are host-transposed.
"""
import os
import sys
import math
from contextlib import ExitStack

os.environ.setdefault("NEURON_RT_RESET_CORES", "1")
sys.path.insert(0, "/opt/trn_rl_repo")

import numpy as np
import concourse.bass as bass
import concourse.tile as tile
from concourse import mybir

F32 = mybir.dt.float32
F32R = mybir.dt.float32r
BF16 = mybir.dt.bfloat16
AX = mybir.AxisListType.X
AF = mybir.ActivationFunctionType
OP = mybir.AluOpType


class Cfg:
    def __init__(self, S=2048, E=1024, H=16, D=64, FE=4, T=512, eps=1e-5):
        self.S, self.E, self.H, self.D, self.FE, self.T, self.eps = S, E, H, D, FE, T, eps
        assert D == 64 and E == H * D
        self.KT = S // 128            # k partition-tiles
        self.QT = T // 128            # q tiles (per core)
        self.ET = E // 128            # e tiles
        self.ZT = FE * E // 128       # ffn hidden tiles
        self.CH = min(512, S)         # k moving chunk for pass1 / k'
        self.NCH = S // self.CH
        self.EC = min(512, E)         # e moving chunk
        self.NEC = E // self.EC
        self.P2B = 2 if self.KT % 2 == 0 else 1   # pass-2 k-tiles per exp batch
        self.scale = math.sqrt(float(S))

    def perm(self):
        # pass-2 query order j <-> original query (j % QT)*128 + j // QT
        j = np.arange(self.T)
        return (j % self.QT) * 128 + j // self.QT


def _layernorm(nc, pool, x_ap, out_ap, g_b, b_b, eps_t, c, out_dtype=None):
    """LayerNorm over the free dim (E) of x_ap [128, E] -> out_ap."""
    E = c.E
    nsub = (E + 511) // 512
    sub = E // nsub
    stats = pool.tile([128, nsub, 6], F32, tag="ln_stats")
    xr = x_ap.rearrange("p (n s) -> p n s", n=nsub)
    for i in range(nsub):
        nc.vector.bn_stats(stats[:, i, :], xr[:, i, :])
    mv = pool.tile([128, 2], F32, tag="ln_mv")
    nc.vector.bn_aggr(mv[:], stats[:])
    rstd = pool.tile([128, 1], F32, tag="ln_rstd")
    nc.scalar.activation(rstd[:], mv[:, 1:2], AF.Sqrt, bias=eps_t[:], scale=1.0)
    nc.vector.reciprocal(rstd[:], rstd[:])
    t1 = pool.tile([128, E], F32, tag="ln_t1")
    nc.vector.scalar_tensor_tensor(
        t1[:], x_ap, mv[:, 0:1], rstd[:].to_broadcast([128, E]),
        OP.subtract, OP.mult)
    t2 = pool.tile([128, E], F32, tag="ln_t2")
    nc.vector.tensor_tensor(t2[:], t1[:], g_b[:], OP.mult)
    nc.vector.tensor_tensor(out_ap, t2[:], b_b[:], OP.add)


def build_nc(c: Cfg):
    """Build the single-core program (pure SPMD — all cores run this)."""
    nc = bass.Bass()
    S, E, H, D, T = c.S, c.E, c.H, c.D, c.T

    dp = nc.declare_dram_parameter
    # k split hi/lo, interleaved per head: [H][0:64]=kh^T, [64:128]=kl^T
    khl_d = dp("khl", [H, 128, S], F32, isOutput=False)
    qh_d = dp("qh", [E, T], F32, isOutput=False)             # q'_hi^T (orig order)
    qhd_d = dp("qhd", [H, 128, T], F32, isOutput=False)      # q'_hi^T dup (perm order)
    ql_d = dp("ql", [E, T], F32, isOutput=False)             # q'_lo^T (perm order)
    qnat_d = dp("qnat", [T, E], F32, isOutput=False)         # queries rows (perm order)
    v_d = dp("vv", [S, E], BF16, isOutput=False)             # values of batch
    wfc_d = dp("wfc", [128, c.ET, E], BF16, isOutput=False)  # Wfc_v^T tiled
    bfc_d = dp("bfc", [1, E], BF16, isOutput=False)
    w1_d = dp("w1", [c.ZT, 128, E], BF16, isOutput=False)    # per zt: [e_in part, z cols]
    b1_d = dp("b1", [128, c.ZT], F32, isOutput=False)
    w2_d = dp("w2", [c.ZT, 128, E], BF16, isOutput=False)    # per zt: [z part, e cols]
    b2_d = dp("b2", [1, E], BF16, isOutput=False)
    g1_d = dp("g1", [1, E], F32, isOutput=False)
    be1_d = dp("be1", [1, E], F32, isOutput=False)
    g2_d = dp("g2", [1, E], F32, isOutput=False)
    be2_d = dp("be2", [1, E], F32, isOutput=False)
    out_d = dp("out", [T, E], F32, isOutput=True)            # perm rows

    with tile.TileContext(nc) as tc, ExitStack() as ctx:
        persist = ctx.enter_context(tc.tile_pool(name="persist", bufs=1))

        def bcast128(src_ap, nm, dtype=F32):
            t = persist.tile([128, src_ap.shape[1]], dtype, name=nm, tag=nm)
            src_b = bass.AP(tensor=src_ap.tensor, offset=src_ap.offset,
                            ap=[[0, 128]] + list(src_ap.ap[1:]))
            nc.sync.dma_start(t[:], src_b)
            return t

        g1_b = bcast128(g1_d[:], "g1b")
        be1_b = bcast128(be1_d[:], "be1b")
        g2_b = bcast128(g2_d[:], "g2b")
        be2_b = bcast128(be2_d[:], "be2b")

        from concourse.masks import make_identity
        ident = persist.tile([128, 128], BF16)
        make_identity(nc, ident[:])

        eps_t = persist.tile([128, 1], F32)
        nc.vector.memset(eps_t[:], c.eps)

        ones_f = persist.tile([1, 512], F32)
        nc.vector.memset(ones_f[:], 1.0)
        ones_bf = persist.tile([1, 128], BF16)
        nc.vector.memset(ones_bf[:], 1.0)

        wfc_t = persist.tile([128, c.ET, E], BF16)
        nc.sync.dma_start(wfc_t[:], wfc_d[:])
        bfc_t = persist.tile([1, E], BF16)
        nc.sync.dma_start(bfc_t[:], bfc_d[:])
        b1_t = persist.tile([128, c.ZT], F32)
        nc.sync.dma_start(b1_t[:], b1_d[:])
        b2_t = persist.tile([1, E], BF16)
        nc.sync.dma_start(b2_t[:], b2_d[:])

        ovT_pack = persist.tile([128, c.ET, T], BF16)
        h_sb = persist.tile([128, c.QT, E], F32)
        hT_bf = persist.tile([128, c.ET, T], BF16)
        x_sb = persist.tile([128, c.QT, E], F32)

        # =================== ATTENTION ===================
        with ExitStack() as actx:
            kst_p = actx.enter_context(tc.tile_pool(name="kst", bufs=1))
            kaug_p = actx.enter_context(tc.tile_pool(name="kaug", bufs=2))
            qtp_p = actx.enter_context(tc.tile_pool(name="qtp", bufs=2))
            qaug_p = actx.enter_context(tc.tile_pool(name="qaug", bufs=2))
            ovs_p = actx.enter_context(tc.tile_pool(name="ovs", bufs=2))
            attn_p = actx.enter_context(tc.tile_pool(name="attn", bufs=2))
            vv_p = actx.enter_context(tc.tile_pool(name="vv", bufs=2))
            sm_p = actx.enter_context(tc.tile_pool(name="sm", bufs=3))
            zi_p = actx.enter_context(tc.tile_pool(name="zi", bufs=2))
            zdr_p = actx.enter_context(tc.tile_pool(name="zdr", bufs=2, space="DRAM"))
            mm_ps = actx.enter_context(tc.tile_pool(name="mm_ps", bufs=2, space="PSUM"))
            p2_ps = actx.enter_context(tc.tile_pool(name="p2_ps", bufs=2, space="PSUM"))
            ov_ps = actx.enter_context(tc.tile_pool(name="ov_ps", bufs=2, space="PSUM"))

            ovst = None
            for h in range(H):
                if h % 2 == 0:
                    ovst = ovs_p.tile([64, 2, T], BF16, tag="ovst")

                # ---- per-head q'_hi rounding (orig order, pass-1 lhsT) ----
                qtstg = qtp_p.tile([64, T], F32, tag="qtstg")
                nc.sync.dma_start(qtstg[:], qh_d[h * D:(h + 1) * D, :])
                qtr = qtp_p.tile([64, T], F32R, tag="qtr")
                nc.vector.tensor_copy(qtr[:], qtstg[:])

                # ---- k hi/lo staging + rounding ----
                k_stage = kst_p.tile([128, S], F32, tag="kst")
                nc.sync.dma_start(k_stage[:], khl_d[h, :, :])
                khl_r = kst_p.tile([128, S], F32R, tag="khlr")
                nc.vector.tensor_copy(khl_r[:], k_stage[:])

                # lhsT-B: [kh; ones] [65, S]
                kaug = kaug_p.tile([65, S], F32R, tag="kaug")
                nc.vector.tensor_copy(kaug[:64, :], k_stage[:64, :])
                nc.scalar.copy(kaug[64:65, :],
                               ones_f[:, 0:1].to_broadcast([1, S]))  # ones row

                # rhs-A: q'_hi duplicated [128, T] (perm order)
                qdstg = qtp_p.tile([128, T], F32, tag="qdstg")
                nc.sync.dma_start(qdstg[:], qhd_d[h, :, :])
                qdup_r = qtp_p.tile([128, T], F32R, tag="qdup")
                nc.vector.tensor_copy(qdup_r[:], qdstg[:])

                # ---- pass 1: per-row max ----
                m_neg = sm_p.tile([128, c.QT], F32, tag="mneg")
                for qt in range(c.QT):
                    mtmp = sm_p.tile([128, max(c.NCH, 2)], F32, tag="mtmp")
                    for j in range(c.NCH):
                        sl = slice(j * c.CH, (j + 1) * c.CH)
                        sps = mm_ps.tile([128, c.CH], F32, tag="mmps")
                        nc.tensor.matmul(
                            sps[:], qtr[:, qt * 128:(qt + 1) * 128],
                            khl_r[:64, sl], start=True, stop=True)
                        nc.vector.reduce_max(mtmp[:, j:j + 1], sps[:], axis=AX)
                    nc.vector.reduce_max(m_neg[:, qt:qt + 1], mtmp[:, :c.NCH], axis=AX)
                nc.vector.tensor_scalar_mul(m_neg[:], m_neg[:], -1.0)

                # ---- qT_aug [65, T]: perm-order queries + (-m) aug row ----
                # (m [128, QT] -> [1, T] free-dim row needs a DRAM bounce:
                #  SBUF APs cannot flatten across partitions)
                m_dram = zdr_p.tile([128, c.QT], F32, tag="mdram")
                nc.sync.dma_start(m_dram[:], m_neg[:])
                q_stage = qaug_p.tile([65, T], F32, tag="qstage")
                nc.sync.dma_start(q_stage[:64, :], ql_d[h * D:(h + 1) * D, :])
                nc.sync.dma_start(q_stage[64:65, :],
                                  m_dram[:].rearrange("r qt -> (r qt)")[None, :])
                qaug = qaug_p.tile([65, T], F32R, tag="qaug")
                nc.vector.tensor_copy(qaug[:], q_stage[:])

                # ---- v_aug [128, KT, 65] (ones column for Z) ----
                vaug = vv_p.tile([128, c.KT, 65], BF16, tag="vaug")
                nc.sync.dma_start(
                    vaug[:, :, :64],
                    v_d[:, h * D:(h + 1) * D].rearrange("(t p) d -> p t d", p=128))
                nc.vector.memset(vaug[:, :, 64:65], 1.0)

                # ---- pass 2 (scoresT - m), exp, ov ----
                attnT = attn_p.tile([128, c.KT, T], BF16, tag="attnT")
                ovp = ov_ps.tile([65, T], F32, tag="ovps")
                for tb in range(0, c.KT, c.P2B):
                    p2 = p2_ps.tile([128, c.P2B, T], F32, tag="p2ps")
                    for ti in range(c.P2B):
                        t = tb + ti
                        tsl = slice(t * 128, (t + 1) * 128)
                        # (kh+kl)*q'hi  then  kh*q'lo + (-m)
                        nc.tensor.matmul(p2[:, ti, :], khl_r[:, tsl], qdup_r[:],
                                         start=True, stop=False)
                        nc.tensor.matmul(p2[:, ti, :], kaug[:, tsl], qaug[:],
                                         start=False, stop=True)
                    nc.scalar.activation(attnT[:, tb:tb + c.P2B, :], p2[:],
                                         AF.Exp, bias=0.0, scale=c.scale)
                    for ti in range(c.P2B):
                        t = tb + ti
                        nc.tensor.matmul(
                            ovp[:], vaug[:, t, :], attnT[:, t, :],
                            start=(t == 0), stop=(t == c.KT - 1),
                            skip_group_check=True)

                # ---- 1/Z and ovT drain ----
                zrow = zi_p.tile([65, T], F32, tag="zrow")
                nc.vector.reciprocal(zrow[64:65, :], ovp[64:65, :])
                zdr = zdr_p.tile([1, T], F32, tag="zdr")
                nc.sync.dma_start(zdr[:], zrow[64:65, :])
                zinv_b = zi_p.tile([64, T], F32, tag="zinv")
                zsrc = zdr[:]
                nc.sync.dma_start(
                    zinv_b[:],
                    bass.AP(tensor=zsrc.tensor, offset=zsrc.offset,
                            ap=[[0, 64]] + list(zsrc.ap[1:])))
                nc.vector.scalar_tensor_tensor(
                    ovst[:, h % 2, :], ovp[:64, :], 1.0, zinv_b[:],
                    OP.bypass, OP.mult)

                if h % 2 == 1:
                    # pack pair -> ovT_pack [128, ET, T]
                    nc.sync.dma_start(ovT_pack[:64, h // 2, :], ovst[:, 0, :])
                    nc.sync.dma_start(ovT_pack[64:128, h // 2, :], ovst[:, 1, :])

        # =================== FC + LN1 + transpose(h) ===================
        with ExitStack() as fctx:
            qn_p = fctx.enter_context(tc.tile_pool(name="qn", bufs=2))
            st_p = fctx.enter_context(tc.tile_pool(name="st", bufs=2))
            fc_ps = fctx.enter_context(tc.tile_pool(name="fc_ps", bufs=2, space="PSUM"))
            tr_ps = fctx.enter_context(tc.tile_pool(name="tr_ps", bufs=2, space="PSUM"))

            for qt in range(c.QT):
                qsl = slice(qt * 128, (qt + 1) * 128)
                hpre = st_p.tile([128, E], F32, tag="hpre")
                qn = qn_p.tile([128, E], F32, tag="qn")
                nc.sync.dma_start(qn[:], qnat_d[qsl, :])
                for ec in range(c.NEC):
                    esl = slice(ec * c.EC, (ec + 1) * c.EC)
                    aps = fc_ps.tile([128, c.EC], F32, tag="fcps")
                    for dt in range(c.ET):
                        nc.tensor.matmul(aps[:], ovT_pack[:, dt, qsl],
                                         wfc_t[:, dt, esl],
                                         start=(dt == 0), stop=False)
                    nc.tensor.matmul(aps[:], ones_bf[:, :128], bfc_t[:, esl],
                                     start=False, stop=True)
                    nc.vector.scalar_tensor_tensor(
                        hpre[:, esl], aps[:], 1.0, qn[:, esl],
                        OP.bypass, OP.add)

                _layernorm(nc, st_p, hpre[:], h_sb[:, qt, :], g1_b, be1_b, eps_t, c)
                hbf = st_p.tile([128, E], BF16, tag="hbf")
                nc.vector.tensor_copy(hbf[:], h_sb[:, qt, :])
                for et in range(c.ET):
                    tps = tr_ps.tile([128, 128], BF16, tag="trps")
                    nc.tensor.transpose(tps[:], hbf[:, et * 128:(et + 1) * 128],
                                        ident[:])
                    nc.vector.tensor_copy(hT_bf[:, et, qsl], tps[:])

        # =================== FFN + LN2 ===================
        with ExitStack() as nctx:
            w_p = nctx.enter_context(tc.tile_pool(name="wstream", bufs=4))
            z_p = nctx.enter_context(tc.tile_pool(name="zrel", bufs=1))
            ln_p = nctx.enter_context(tc.tile_pool(name="lnp", bufs=2))
            z1_ps = nctx.enter_context(tc.tile_pool(name="z1_ps", bufs=2, space="PSUM"))
            x2_ps = nctx.enter_context(
                tc.tile_pool(name="x2_ps", bufs=c.QT, space="PSUM"))

            z1rel = z_p.tile([128, c.ZT, T], BF16, tag="z1rel")
            for zt in range(c.ZT):
                w1t = w_p.tile([128, E], BF16, tag="w1t")
                nc.sync.dma_start(w1t[:], w1_d[zt, :, :])
                zps = z1_ps.tile([128, T], F32, tag="z1ps")
                for et in range(c.ET):
                    nc.tensor.matmul(zps[:], w1t[:, et * 128:(et + 1) * 128],
                                     hT_bf[:, et, :],
                                     start=(et == 0), stop=(et == c.ET - 1))
                nc.scalar.activation(z1rel[:, zt, :], zps[:], AF.Relu,
                                     bias=b1_t[:, zt:zt + 1], scale=1.0)

            for ec in range(c.NEC):
                esl = slice(ec * c.EC, (ec + 1) * c.EC)
                xps = [x2_ps.tile([128, c.EC], F32, tag="x2ps", name=f"x2ps_{qt}")
                       for qt in range(c.QT)]
                for zt in range(c.ZT):
                    w2t = w_p.tile([128, c.EC], BF16, tag="w2t")
                    nc.sync.dma_start(w2t[:], w2_d[zt, :, esl])
                    for qt in range(c.QT):
                        nc.tensor.matmul(
                            xps[qt][:], z1rel[:, zt, qt * 128:(qt + 1) * 128],
                            w2t[:], start=(zt == 0), stop=False)
                for qt in range(c.QT):
                    nc.tensor.matmul(xps[qt][:], ones_bf[:, :128], b2_t[:, esl],
                                     start=False, stop=True)
                    nc.vector.scalar_tensor_tensor(
                        x_sb[:, qt, esl], xps[qt][:], 1.0, h_sb[:, qt, esl],
                        OP.bypass, OP.add)

            for qt in range(c.QT):
                outt = ln_p.tile([128, E], F32, tag="outt")
                _layernorm(nc, ln_p, x_sb[:, qt, :], outt[:], g2_b, be2_b, eps_t, c)
                nc.sync.dma_start(out_d[qt * 128:(qt + 1) * 128, :], outt[:])

    return nc


def _split_waits(nc, maxw=1):
    """walrus in this toolchain only accepts 1 sync-wait per instruction on
    several formats; move excess waits onto preceding same-engine NoOps."""
    ctr = 0
    for f in nc.m.functions:
        for bb in f.blocks:
            out = []
            for inst in bb.instructions:
                si = getattr(inst, "sync_info", None)
                if si is not None and si.on_wait and len(si.on_wait) > maxw:
                    waits = list(si.on_wait)
                    head, tail = waits[:-maxw], waits[-maxw:]
                    for i in range(0, len(head), maxw):
                        ctr += 1
                        out.append(mybir.InstNoOp(
                            name=f"waitsplit_{ctr}", engine=inst.engine,
                            ins=[], outs=[],
                            sync_info=mybir.SyncInfo(
                                on_wait=list(head[i:i + maxw]), on_update=[]),
                        ))
                    si.on_wait = tail
                out.append(inst)
            bb.instructions[:] = out


# ======================= host side =======================

def host_prep(c: Cfg, inputs, core):
    """Build the per-core input map (numpy only)."""
    B = inputs["queries"].shape[0]
    cores_per_batch = 8 // B if B <= 8 else 1
    b = core // cores_per_batch
    slot = core % cores_per_batch
    T = c.T
    perm = c.perm()

    q = np.asarray(inputs["queries"][b], np.float32)       # [S, E]
    k = np.asarray(inputs["keys"][b], np.float32)
    v = np.asarray(inputs["values"][b], np.float32)
    qs = q[slot * T:(slot + 1) * T]                        # [T, E]

    Wq = np.asarray(inputs["Wq"], np.float64)
    Wk = np.asarray(inputs["Wk"], np.float64)
    Wv = np.asarray(inputs["Wv"], np.float64)
    Wfc = np.asarray(inputs["Wfc"], np.float64)            # [E, E]
    W1 = np.asarray(inputs["W1"], np.float64)              # [FE*E, E]
    W2 = np.asarray(inputs["W2"], np.float64)              # [E, FE*E]

    def trunc10(x):
        # keep 10 mantissa bits -> exactly representable in fp32r
        u = x.astype(np.float32).view(np.uint32) & np.uint32(0xFFFFE000)
        return u.view(np.float32)

    # fold Wq/Wk into the queries: q'_h = q_h @ (Wq.T @ Wk); scores = q' @ k^T
    A_mid = Wq.T @ Wk
    E_, H_, D_ = c.E, c.H, c.D
    qp = np.empty((T, E_), np.float64)
    for h in range(H_):
        qp[:, h * D_:(h + 1) * D_] = qs[:, h * D_:(h + 1) * D_].astype(np.float64) @ A_mid
    qp = qp.astype(np.float32)
    qp_h = trunc10(qp)
    qp_l = (qp - qp_h).astype(np.float32)
    k_h = trunc10(k)
    k_l = (k - k_h).astype(np.float32)

    khl = np.empty((c.H, 128, c.S), np.float32)
    for h in range(H_):
        khl[h, :64] = k_h[:, h * D_:(h + 1) * D_].T
        khl[h, 64:] = k_l[:, h * D_:(h + 1) * D_].T

    qhT = np.ascontiguousarray(qp_h.T)                     # [E, T] orig order
    qhTp = qp_h[perm].T                                    # [E, T] perm order
    qhd = np.empty((c.H, 128, T), np.float32)
    for h in range(H_):
        qhd[h, :64] = qhTp[h * D_:(h + 1) * D_]
        qhd[h, 64:] = qhTp[h * D_:(h + 1) * D_]
    qlTp = np.ascontiguousarray(qp_l[perm].T)              # [E, T] perm order

    # Wfc_v[e, h*64+d] = sum_dd Wfc[e, h*64+dd] * Wv[dd, d]
    E, H, D = c.E, c.H, c.D
    wfcv = np.empty((E, E), np.float64)
    for h in range(H):
        wfcv[:, h * D:(h + 1) * D] = Wfc[:, h * D:(h + 1) * D] @ Wv
    # rhs tiles: wfc_prep[p, dt, e] = Wfc_v[e, dt*128+p]
    wfc_prep = np.ascontiguousarray(
        wfcv.T.reshape(c.ET, 128, E).transpose(1, 0, 2)).astype(ml_bf16())

    # w1_prep[zt, p, et*128 + z] = W1[zt*128+z, et*128+p]
    w1r = W1.reshape(c.ZT, 128, c.ET, 128)                 # [zt, z, et, p]
    w1_prep = np.ascontiguousarray(
        w1r.transpose(0, 3, 2, 1).reshape(c.ZT, 128, E)).astype(ml_bf16())

    # w2_prep[zt, p, e] = W2[e, zt*128+p]
    w2r = W2.T.reshape(c.ZT, 128, E)                       # [zt, p, e]
    w2_prep = np.ascontiguousarray(w2r).astype(ml_bf16())

    b1 = np.asarray(inputs["b1"], np.float32)
    b1_prep = np.ascontiguousarray(b1.reshape(c.ZT, 128).T)  # [128, ZT]

    return {
        "khl": khl,
        "qh": qhT,
        "qhd": qhd,
        "ql": qlTp,
        "qnat": np.ascontiguousarray(qs[perm]),
        "vv": v.astype(ml_bf16()),
        "wfc": wfc_prep,
        "bfc": np.asarray(inputs["bfc"], np.float32)[None, :].astype(ml_bf16()),
        "w1": w1_prep,
        "b1": b1_prep,
        "w2": w2_prep,
        "b2": np.asarray(inputs["b2"], np.float32)[None, :].astype(ml_bf16()),
        "g1": np.asarray(inputs["ln1_g"], np.float32)[None, :],
        "be1": np.asarray(inputs["ln1_b"], np.float32)[None, :],
        "g2": np.asarray(inputs["ln2_g"], np.float32)[None, :],
        "be2": np.asarray(inputs["ln2_b"], np.float32)[None, :],
    }


def ml_bf16():
    import ml_dtypes
    return ml_dtypes.bfloat16


_CACHE = {}


def kernel(**inputs):
    """Full-input entry point: shard across 8 cores, run, gather."""
    c = Cfg()
    B, S, E = inputs["queries"].shape
    assert (B, S, E) == (2, c.S, c.E), (B, S, E)

    if "nc" not in _CACHE:
        nc = build_nc(c)
        _split_waits(nc)   # walrus wait-slot workaround (compile path only)
        _CACHE["nc"] = nc
    nc = _CACHE["nc"]

    in_maps = [host_prep(c, inputs, core) for core in range(8)]

    from concourse.bass_utils import run_bass_kernel_spmd
    res = run_bass_kernel_spmd(nc, in_maps, core_ids=list(range(8)))

    perm = c.perm()
    out = np.empty((B, S, E), np.float32)
    cores_per_batch = 4
    for core in range(8):
        b = core // cores_per_batch
        slot = core % cores_per_batch
        block = np.empty((c.T, E), np.float32)
        block[perm] = res.results[core]["out"]
        out[b, slot * c.T:(slot + 1) * c.T] = block
    return out

